# revision 10
# baseline (speedup 1.0000x reference)
"""Trainium2 Bass kernel for nn_AttentionModule (sparse_attention).

Reference math (per batch b):
    scores[l,q]  = sum_d ctx[d,l] * query[d,q]          # (L=1024, Q=256), D=128
    attn_q       = softmax_q(scores)                     # over q (free dim)
    attn_c[q,l]  = softmax_l(4 * attn_q[l,q])            # over l
    wc[d,q]      = sum_l ctx[d,l] * attn_c[q,l]
    returns (wc (B,D,Q), attn_c.reshape(B,Q,32,32))

Sharding: pure data parallel, batch 128 -> 16 per core x 8 cores.

Per-batch on-chip pipeline (all fp32):
  S1: scores chunks (l=128, q=256) via 8 PE matmuls (natural layouts);
      E1 = exp(scores) on ACT with accum_out -> s1 (softmax-1 sum; max
      subtraction skipped: |scores| <~ 70 is safe in fp32);
      r4 = 4/s1;  E1 *= r4 (GPSIMD, broadcast);  E2 = exp(E1) (one big ACT op);
      ctxT chunks via 8 PE transposes.
  S2: wcT[q,(d|1)] = sum_l E2[l,q] * [ctxT[l,d] | 1] -- 16 PE matmuls with a
      ones column appended to ctxT, so column 128 accumulates S2[q] (the
      softmax-2 denominator) for free, in q-partition layout;
      r2 = 1/S2 (cheap per-partition reciprocal);
      E2T via 16 PE transposes; attn_c = E2T * r2 fused into the PSUM->SBUF
      copies (per-partition scalar); wc = transpose(wcT * r2).
Emission is skewed (S1(b+1) before S2(b)) so the in-order PE stream always
has independent work while ACT computes batch b's exponentials.
"""

import numpy as np

import concourse.bass as bass
import concourse.mybir as mybir
import concourse.tile as tile
from concourse import bacc
from concourse.bass_utils import run_bass_kernel_spmd
from concourse.masks import make_identity

N_CORES = 8
B_FULL = 128
B = B_FULL // N_CORES  # 16 batches per core
D = 128
Q = 256
H = 32
W = 32
L = H * W  # 1024
NL = L // 128  # 8 l-chunks
F32 = mybir.dt.float32
BF16 = mybir.dt.bfloat16
EXP = mybir.ActivationFunctionType.Exp


def emit_core_program(nc, q_d, c_d, wc_d, am_d):
    """Emit the per-core program. APs:
    q_d (B,128,256) in, c_d (B,128,1024) in, wc_d (B,128,256) out,
    am_d (B,128,2,1024) out (partition-major view of (B,256,1024))."""
    with tile.TileContext(nc) as tc:
        with (
            tc.tile_pool(name="consts", bufs=1) as consts,
            tc.tile_pool(name="io", bufs=2) as io,
            tc.tile_pool(name="work", bufs=2) as work,
            tc.tile_pool(name="outb", bufs=2) as outb,
            tc.tile_pool(name="stats", bufs=2) as stats,
            tc.tile_pool(name="ps_sc", bufs=2, space="PSUM") as ps_sc,
            tc.tile_pool(name="ps_ct", bufs=2, space="PSUM") as ps_ct,
            tc.tile_pool(name="ps_et", bufs=2, space="PSUM") as ps_et,
            tc.tile_pool(name="ps_wc", bufs=2, space="PSUM") as ps_wc,
        ):
            ident = consts.tile([128, 128], F32)
            make_identity(nc, ident)

            # cross-stage state for the 1-batch software pipeline skew
            st = {}

            def stage1(b):
                ctx_sb = io.tile([128, L], F32, tag="ctx")
                nc.sync.dma_start(out=ctx_sb, in_=c_d[b])
                qry_sb = io.tile([128, Q], F32, tag="qry")
                nc.sync.dma_start(out=qry_sb, in_=q_d[b])

                E1 = work.tile([128, NL, Q], F32, tag="E1")
                E2 = work.tile([128, NL, Q], F32, tag="E2")
                s1 = stats.tile([128, NL], F32, tag="s1")
                r4 = stats.tile([128, NL], F32, tag="r4")
                nm = stats.tile([128, NL], F32, tag="nm")

                # scores (2 chunks per PSUM bank); softmax-1 with max
                # subtraction (scores reach ~±300 on real data) — negated
                # row-max feeds the exp as its per-partition bias.
                for g in range(NL // 2):
                    sc = ps_sc.tile([128, 2, Q], F32, tag="sc")
                    for j in range(2):
                        c = 2 * g + j
                        nc.tensor.matmul(
                            sc[:, j],
                            lhsT=ctx_sb[:, c * 128 : (c + 1) * 128],
                            rhs=qry_sb,
                            start=True,
                            stop=True,
                        )
                    nc.vector.tensor_reduce(
                        out=nm[:, 2 * g : 2 * g + 2],
                        in_=sc[:],
                        axis=mybir.AxisListType.X,
                        op=mybir.AluOpType.max,
                        negate=True,
                    )
                    for j in range(2):
                        c = 2 * g + j
                        nc.scalar.activation(
                            out=E1[:, c],
                            in_=sc[:, j],
                            func=EXP,
                            bias=nm[:, c : c + 1],
                            accum_out=s1[:, c : c + 1],
                        )
                # r4 = 4 / s1
                nc.vector.reciprocal(out=r4, in_=s1)
                nc.vector.tensor_scalar_mul(r4, r4, 4.0)
                # E1 *= r4 (per l-chunk broadcast along q) on GPSIMD
                nc.gpsimd.tensor_tensor(
                    out=E1[:],
                    in0=E1[:],
                    in1=r4[:, :, None].to_broadcast((128, NL, Q)),
                    op=mybir.AluOpType.mult,
                )
                # E2 = exp(4 * attn_q), one big ACT op
                nc.scalar.activation(out=E2[:], in_=E1[:], func=EXP)
                # bf16 copy of E2 for the wc matmul (fp32 E2 is kept for the
                # exact attn_map path); GPSIMD has idle capacity
                E2bf = work.tile([128, NL, Q], BF16, tag="E2bf")
                nc.gpsimd.tensor_copy(out=E2bf[:], in_=E2[:])

                # ctxT chunks in bf16 (with ones column at 128 for the S2
                # trick) — feeds only the bf16 wc matmul
                ctxT = work.tile([128, NL, 132], BF16, tag="ctxT")
                nc.vector.memset(ctxT[:, :, 128:129], 1.0)
                for g in range(2):
                    ct = ps_ct.tile([128, 4, 128], F32, tag="ct")
                    for j in range(4):
                        c = 4 * g + j
                        nc.tensor.transpose(
                            ct[:, j], ctx_sb[:, c * 128 : (c + 1) * 128], ident
                        )
                    nc.scalar.copy(
                        out=ctxT[:, 4 * g : 4 * g + 4, :128], in_=ct[:]
                    )
                st[b] = (ctx_sb, qry_sb, E2, E2bf, ctxT)

            def stage2(b):
                ctx_sb, qry_sb, E2, E2bf, ctxT = st.pop(b)
                # wcT (+ fused S2 in column 128)
                r2s = []
                wcT_sbs = []
                for qh in range(2):
                    wcT = ps_wc.tile([128, 129], F32, tag="wcT")
                    for c in range(NL):
                        nc.tensor.matmul(
                            wcT,
                            lhsT=E2bf[:, c, qh * 128 : (qh + 1) * 128],
                            rhs=ctxT[:, c, :129],
                            start=(c == 0),
                            stop=(c == NL - 1),
                        )
                    r2 = stats.tile([128, 1], F32, tag=f"r2_{qh}")
                    nc.vector.reciprocal(out=r2, in_=wcT[:, 128:129])
                    wcT_sb = work.tile([128, 128], F32, tag=f"wcTsb{qh}")
                    nc.scalar.mul(out=wcT_sb, in_=wcT[:, :128], mul=r2)
                    r2s.append(r2)
                    wcT_sbs.append(wcT_sb)

                # attn_c = transpose(E2) * r2 (scale fused into PSUM->SBUF copy)
                attnc = outb.tile([128, 2, L], F32, tag="attnc")
                for qh in range(2):
                    for g in range(2):
                        et = ps_et.tile([128, 4, 128], F32, tag="et")
                        for j in range(4):
                            c = 4 * g + j
                            nc.tensor.transpose(
                                et[:, j],
                                E2[:, c, qh * 128 : (qh + 1) * 128],
                                ident,
                            )
                        nc.vector.tensor_scalar_mul(
                            attnc[:, qh, g * 512 : (g + 1) * 512],
                            et[:].rearrange("p a b -> p (a b)"),
                            r2s[qh],
                        )
                # wc = transpose(wcT_sb)
                wcp = ps_et.tile([128, 4, 128], F32, tag="et")
                for qh in range(2):
                    nc.tensor.transpose(wcp[:, qh], wcT_sbs[qh], ident)
                wc_sb = outb.tile([128, Q], F32, tag="wc")
                nc.scalar.copy(
                    out=wc_sb, in_=wcp[:, :2].rearrange("p a b -> p (a b)")
                )
                nc.sync.dma_start(out=am_d[b], in_=attnc[:])
                nc.sync.dma_start(out=wc_d[b], in_=wc_sb)

            for i in range(B + 1):
                if i < B:
                    stage1(i)
                if i > 0:
                    stage2(i - 1)


def build_program():
    nc = bacc.Bacc("TRN2", target_bir_lowering=False, debug=False)
    q_t = nc.dram_tensor("query", (B, D, Q), F32, kind="ExternalInput")
    c_t = nc.dram_tensor("context", (B, D, H, W), F32, kind="ExternalInput")
    wc_t = nc.dram_tensor("wc", (B, D, Q), F32, kind="ExternalOutput")
    am_t = nc.dram_tensor("attn_map", (B, Q, H, W), F32, kind="ExternalOutput")

    q_d = q_t.ap()
    c_d = c_t.ap().rearrange("b d h w -> b d (h w)")
    wc_d = wc_t.ap()
    # (B, Q, L) viewed partition-major: q = s*128 + p  ->  (B, 128, 2, L)
    am_d = am_t.ap().rearrange("b (s p) h w -> b p s (h w)", p=128)

    emit_core_program(nc, q_d, c_d, wc_d, am_d)
    nc.compile()
    return nc


_CACHED_NC = None


def _run(query, context, trace=False):
    global _CACHED_NC
    if _CACHED_NC is None:
        _CACHED_NC = build_program()
    nc = _CACHED_NC

    query = np.ascontiguousarray(np.asarray(query, dtype=np.float32))
    context = np.ascontiguousarray(np.asarray(context, dtype=np.float32))
    assert query.shape == (B_FULL, D, Q), query.shape
    assert context.shape == (B_FULL, D, H, W), context.shape

    in_maps = [
        {
            "query": query[i * B : (i + 1) * B],
            "context": context[i * B : (i + 1) * B],
        }
        for i in range(N_CORES)
    ]
    res = run_bass_kernel_spmd(
        nc, in_maps, core_ids=list(range(N_CORES)), trace=trace
    )
    wc = np.concatenate([r["wc"] for r in res.results], axis=0)
    am = np.concatenate([r["attn_map"] for r in res.results], axis=0)
    return (wc, am), res


def kernel(query, context):
    (wc, am), _ = _run(query, context, trace=False)
    return wc, am


# revision 14
# speedup vs baseline: 1.6306x; 1.6306x over previous
"""Trainium2 Bass kernel for nn_AttentionModule (sparse_attention).

Reference math (per batch b):
    scores[l,q]  = sum_d ctx[d,l] * query[d,q]          # (L=1024, Q=256), D=128
    attn_q       = softmax_q(scores)                     # over q (free dim)
    attn_c[q,l]  = softmax_l(4 * attn_q[l,q])            # over l
    wc[d,q]      = sum_l ctx[d,l] * attn_c[q,l]
    returns (wc (B,D,Q), attn_c.reshape(B,Q,32,32))

Sharding: pure data parallel, batch 128 -> 16 per core x 8 cores.

Per-batch on-chip pipeline (all fp32):
  S1: scores chunks (l=128, q=256) via 8 PE matmuls (natural layouts);
      E1 = exp(scores) on ACT with accum_out -> s1 (softmax-1 sum; max
      subtraction skipped: |scores| <~ 70 is safe in fp32);
      r4 = 4/s1;  E1 *= r4 (GPSIMD, broadcast);  E2 = exp(E1) (one big ACT op);
      ctxT chunks via 8 PE transposes.
  S2: wcT[q,(d|1)] = sum_l E2[l,q] * [ctxT[l,d] | 1] -- 16 PE matmuls with a
      ones column appended to ctxT, so column 128 accumulates S2[q] (the
      softmax-2 denominator) for free, in q-partition layout;
      r2 = 1/S2 (cheap per-partition reciprocal);
      E2T via 16 PE transposes; attn_c = E2T * r2 fused into the PSUM->SBUF
      copies (per-partition scalar); wc = transpose(wcT * r2).
Emission is skewed (S1(b+1) before S2(b)) so the in-order PE stream always
has independent work while ACT computes batch b's exponentials.
"""

import numpy as np

import concourse.bass as bass
import concourse.mybir as mybir
import concourse.tile as tile
from concourse import bacc
from concourse.bass_utils import run_bass_kernel_spmd
from concourse.masks import make_identity

N_CORES = 8
B_FULL = 128
B = B_FULL // N_CORES  # 16 batches per core
D = 128
Q = 256
H = 32
W = 32
L = H * W  # 1024
NL = L // 128  # 8 l-chunks
F32 = mybir.dt.float32
BF16 = mybir.dt.bfloat16
EXP = mybir.ActivationFunctionType.Exp

# bf16 wc-matmul variant: ~3µs/batch faster on PE but wc rel err ~2e-3
# (vs ~6e-6 all-fp32). Default off — accuracy first.
WCT_BF16 = False


def emit_core_program(nc, q_d, c_d, wc_d, am_d):
    """Emit the per-core program. APs:
    q_d (B,128,256) in, c_d (B,128,1024) in, wc_d (B,128,256) out,
    am_d (B,128,2,1024) out (partition-major view of (B,256,1024))."""
    with tile.TileContext(nc) as tc:
        with (
            tc.tile_pool(name="consts", bufs=1) as consts,
            tc.tile_pool(name="io", bufs=2) as io,
            tc.tile_pool(name="work", bufs=2) as work,
            tc.tile_pool(name="outb", bufs=2) as outb,
            tc.tile_pool(name="stats", bufs=2) as stats,
            tc.tile_pool(name="ps_sc", bufs=2, space="PSUM") as ps_sc,
            tc.tile_pool(name="ps_ct", bufs=2, space="PSUM") as ps_ct,
            tc.tile_pool(name="ps_et", bufs=2, space="PSUM") as ps_et,
            tc.tile_pool(name="ps_wc", bufs=2, space="PSUM") as ps_wc,
        ):
            ident = consts.tile([128, 128], F32)
            make_identity(nc, ident)

            # cross-stage state for the 1-batch software pipeline skew
            st = {}

            def stage1(b):
                E2dt = BF16 if WCT_BF16 else F32
                ctx_sb = io.tile([128, L], F32, tag="ctx")
                nc.sync.dma_start(out=ctx_sb, in_=c_d[b])
                qry_sb = io.tile([128, Q], F32, tag="qry")
                nc.sync.dma_start(out=qry_sb, in_=q_d[b])

                E1 = work.tile([128, NL, Q], F32, tag="E1")
                E2 = work.tile([128, NL, Q], F32, tag="E2")
                s1 = stats.tile([128, NL], F32, tag="s1")
                r4 = stats.tile([128, NL], F32, tag="r4")
                nm = stats.tile([128, NL], F32, tag="nm")

                # scores (2 chunks per PSUM bank); softmax-1 with max
                # subtraction (scores reach ~±300 on real data) — negated
                # row-max feeds the exp as its per-partition bias. Stats and
                # both exps run per-chunk so downstream PE work (wcT matmuls,
                # E2T transposes) unblocks progressively instead of waiting
                # for a whole-batch ACT chain.
                for g in range(NL // 2):
                    sc = ps_sc.tile([128, 2, Q], F32, tag="sc")
                    for j in range(2):
                        c = 2 * g + j
                        nc.tensor.matmul(
                            sc[:, j],
                            lhsT=ctx_sb[:, c * 128 : (c + 1) * 128],
                            rhs=qry_sb,
                            start=True,
                            stop=True,
                        )
                    nc.vector.tensor_reduce(
                        out=nm[:, 2 * g : 2 * g + 2],
                        in_=sc[:],
                        axis=mybir.AxisListType.X,
                        op=mybir.AluOpType.max,
                        negate=True,
                    )
                    for j in range(2):
                        c = 2 * g + j
                        nc.scalar.activation(
                            out=E1[:, c],
                            in_=sc[:, j],
                            func=EXP,
                            bias=nm[:, c : c + 1],
                            accum_out=s1[:, c : c + 1],
                        )
                    # r4 = 4 / s1 for this pair
                    nc.vector.reciprocal(
                        out=r4[:, 2 * g : 2 * g + 2],
                        in_=s1[:, 2 * g : 2 * g + 2],
                    )
                    nc.vector.tensor_scalar_mul(
                        r4[:, 2 * g : 2 * g + 2], r4[:, 2 * g : 2 * g + 2], 4.0
                    )
                    for j in range(2):
                        c = 2 * g + j
                        nc.scalar.activation(
                            out=E2[:, c],
                            in_=E1[:, c],
                            func=EXP,
                            scale=r4[:, c : c + 1],
                        )

                # ctxT chunks (ones column at 128 feeds the fused-S2 trick);
                # PE filler while ACT runs the exp chain
                ctxT = work.tile([128, NL, 132], E2dt, tag="ctxT")
                nc.vector.memset(ctxT[:, :, 128:129], 1.0)
                for g in range(2):
                    ct = ps_ct.tile([128, 4, 128], F32, tag="ct")
                    for j in range(4):
                        c = 4 * g + j
                        nc.tensor.transpose(
                            ct[:, j], ctx_sb[:, c * 128 : (c + 1) * 128], ident
                        )
                    nc.scalar.copy(
                        out=ctxT[:, 4 * g : 4 * g + 4, :128], in_=ct[:]
                    )

                if WCT_BF16:
                    E2w = work.tile([128, NL, Q], BF16, tag="E2bf")
                    nc.gpsimd.tensor_copy(out=E2w[:], in_=E2[:])
                else:
                    E2w = E2

                # wcT (+ fused S2 in column 128), chunk-major so each matmul
                # needs only E2[c]
                wcTs = [
                    ps_wc.tile([128, 129], F32, tag="wcT", name=f"wcT{qh}")
                    for qh in range(2)
                ]
                for c in range(NL):
                    for qh in range(2):
                        nc.tensor.matmul(
                            wcTs[qh],
                            lhsT=E2w[:, c, qh * 128 : (qh + 1) * 128],
                            rhs=ctxT[:, c, :129],
                            start=(c == 0),
                            stop=(c == NL - 1),
                        )
                r2s = []
                wcT_sbs = []
                for qh in range(2):
                    r2 = stats.tile([128, 1], F32, tag=f"r2_{qh}")
                    nc.vector.reciprocal(out=r2, in_=wcTs[qh][:, 128:129])
                    wcT_sb = work.tile([128, 128], F32, tag=f"wcTsb{qh}")
                    nc.scalar.mul(out=wcT_sb, in_=wcTs[qh][:, :128], mul=r2)
                    r2s.append(r2)
                    wcT_sbs.append(wcT_sb)

                # attn_c = transpose(E2) * r2 (scale fused into PSUM->SBUF
                # copy); evacuated within the batch to keep PSUM pressure low
                attnc = outb.tile([128, 2, L], F32, tag="attnc")
                for qh in range(2):
                    for g in range(2):
                        et = ps_et.tile([128, 4, 128], F32, tag="et")
                        for j in range(4):
                            c = 4 * g + j
                            nc.tensor.transpose(
                                et[:, j],
                                E2[:, c, qh * 128 : (qh + 1) * 128],
                                ident,
                            )
                        nc.vector.tensor_scalar_mul(
                            attnc[:, qh, g * 512 : (g + 1) * 512],
                            et[:].rearrange("p a b -> p (a b)"),
                            r2s[qh],
                        )
                st[b] = (wcT_sbs, attnc)

            def stage2(b):
                wcT_sbs, attnc = st.pop(b)
                # wc = transpose(wcT_sb); runs during batch b+1's compute
                wcp = ps_ct.tile([128, 4, 128], F32, tag="ct")
                for qh in range(2):
                    nc.tensor.transpose(wcp[:, qh], wcT_sbs[qh], ident)
                wc_sb = outb.tile([128, Q], F32, tag="wc")
                nc.vector.tensor_copy(
                    out=wc_sb, in_=wcp[:, :2].rearrange("p a b -> p (a b)")
                )
                nc.sync.dma_start(out=am_d[b], in_=attnc[:])
                nc.sync.dma_start(out=wc_d[b], in_=wc_sb)

            for i in range(B + 1):
                if i < B:
                    stage1(i)
                if i > 0:
                    stage2(i - 1)


def build_program():
    nc = bacc.Bacc("TRN2", target_bir_lowering=False, debug=False)
    q_t = nc.dram_tensor("query", (B, D, Q), F32, kind="ExternalInput")
    c_t = nc.dram_tensor("context", (B, D, H, W), F32, kind="ExternalInput")
    wc_t = nc.dram_tensor("wc", (B, D, Q), F32, kind="ExternalOutput")
    am_t = nc.dram_tensor("attn_map", (B, Q, H, W), F32, kind="ExternalOutput")

    q_d = q_t.ap()
    c_d = c_t.ap().rearrange("b d h w -> b d (h w)")
    wc_d = wc_t.ap()
    # (B, Q, L) viewed partition-major: q = s*128 + p  ->  (B, 128, 2, L)
    am_d = am_t.ap().rearrange("b (s p) h w -> b p s (h w)", p=128)

    emit_core_program(nc, q_d, c_d, wc_d, am_d)
    nc.compile()
    return nc


_CACHED_NC = None


def _run(query, context, trace=False):
    global _CACHED_NC
    if _CACHED_NC is None:
        _CACHED_NC = build_program()
    nc = _CACHED_NC

    query = np.ascontiguousarray(np.asarray(query, dtype=np.float32))
    context = np.ascontiguousarray(np.asarray(context, dtype=np.float32))
    assert query.shape == (B_FULL, D, Q), query.shape
    assert context.shape == (B_FULL, D, H, W), context.shape

    in_maps = [
        {
            "query": query[i * B : (i + 1) * B],
            "context": context[i * B : (i + 1) * B],
        }
        for i in range(N_CORES)
    ]
    res = run_bass_kernel_spmd(
        nc, in_maps, core_ids=list(range(N_CORES)), trace=trace
    )
    wc = np.concatenate([r["wc"] for r in res.results], axis=0)
    am = np.concatenate([r["attn_map"] for r in res.results], axis=0)
    return (wc, am), res


def kernel(query, context):
    (wc, am), _ = _run(query, context, trace=False)
    return wc, am
